# revision 29
# baseline (speedup 1.0000x reference)
"""Trainium2 Bass kernel for nn_ODEFunc_interaction (gnn_message_passing).

Math (see reference):
  dz_dt = tanh([z, t] @ vW1 + vb1) @ vW2 + vb2                    (v-net, all rows)
  for each pair (perm[2i], perm[2i+1]):
      d_i  = z[perm[2i]] - z[perm[2i+1]]
      g_i  = grad_phi(d_i) = pW1 @ (pW2[:,0] * (1 - tanh(d_i@pW1 + pb1)^2))
      out[perm[2i]]   = dz_dt[perm[2i]]   - g_i
      out[perm[2i+1]] = dz_dt[perm[2i+1]] + g_i
  last 3 rows (triple) + 53 ragged pairs/chunk handled on host (tiny).

Strategy: host gathers z[perm] so each of 8 cores owns 25000 rows (12500
pairs).  On-device layout is transposed+packed: X[128, 6144] fp16 where
partition 32*j+d holds dim d of row-chunk j (4 chunks x 3072 pairs).
Columns are grouped in superblocks of 512 pairs per chunk: 512 even
members then 512 odd members, so the pair-difference and the final +/-
combine run on contiguous ranges (DVE 4x fp16) and every tanh is one
contiguous 1024-col activation from bank-aligned PSUM (matmul PSUM
outputs must start at a bank boundary - mid-bank starts are fatal).
Per superblock: 8 h-matmuls -> 4 fused tanh -> 8 dz-matmuls; 4
pa-matmuls -> 2 fused tanh -> square -> 4 q-matmuls; DVE combines
+/-q into the dz halves; DMA out.  All matmuls fp16 (1 cyc/col).
Emission is software-pipelined one superblock deep so the activation
engine (the throughput bound, ~36.9k cols/core) never waits on the PE.
PSUM: one 3-slot rotating pool serves h/dz/qp tiles + a 1-slot pa pool
= exactly 8 banks.  Host scatters the result back by perm.
"""

import os
import numpy as np

B, D, H = 200003, 32, 128
NCORES = 8
P2 = 200000            # rows covered by pairs
RPC = P2 // NCORES     # 25000 rows per core
NCHUNK = 4
CH = RPC // NCHUNK     # 6250 rows per chunk
NPAIR = CH // 2        # 3125 pairs per chunk
WPB = 512              # pairs per superblock
NSB = NPAIR // WPB     # 6 full superblocks on device
DPAIR = NSB * WPB      # 3072 pairs per chunk on device
DROW = 2 * DPAIR       # 6144 device rows per chunk
NCOL = DROW            # 6144 packed columns per chunk strip

_CACHE = {}
LAST_RESULTS = None    # BassKernelResults of the most recent run (for test.py)


def build_program():
    """Build the single-core Bass/Tile program (same program runs SPMD on 8 cores)."""
    from contextlib import ExitStack
    import concourse.bacc as bacc
    import concourse.mybir as mybir
    import concourse.tile as tile

    dt = mybir.dt
    F32 = dt.float32
    F16 = dt.float16
    AF = mybir.ActivationFunctionType
    OP = mybir.AluOpType

    # Weights [128, 320] fp16: w1rep[0:128] | pw1rep[128:256] | vw2[256:288]
    # | pwtw2[288:320].  Layer-1 matmuls run as four K=32 row-tiles
    # (tile_position=(32j, 0)), layer-2 as four K=128/M=32 column-tiles
    # (tile_position=(0, 32j)) - consecutive matmuls on disjoint PE array
    # tiles compose into nearly one stream pass, and the column-tiled
    # layer-2 writes each chunk's 32 dims directly to psum partitions 32j.
    nc = bacc.Bacc()
    X = nc.dram_tensor("x", [128, NCOL], F16, kind="ExternalInput")
    WC = nc.dram_tensor("wcat", [128, 320], F16, kind="ExternalInput")
    BC = nc.dram_tensor("bias", [128, 2], F32, kind="ExternalInput")
    O = nc.dram_tensor("out", [128, NCOL], F32, kind="ExternalOutput")

    with tile.TileContext(nc) as tc, ExitStack() as ctx:
        wpool = ctx.enter_context(tc.tile_pool(name="wpool", bufs=1))
        xpool = ctx.enter_context(tc.tile_pool(name="xpool", bufs=3))
        dfpool = ctx.enter_context(tc.tile_pool(name="dfpool", bufs=2))
        utpool = ctx.enter_context(tc.tile_pool(name="utpool", bufs=4))
        vtpool = ctx.enter_context(tc.tile_pool(name="vtpool", bufs=2))
        sqpool = ctx.enter_context(tc.tile_pool(name="sqpool", bufs=3))
        opool = ctx.enter_context(tc.tile_pool(name="opool", bufs=4))
        qspool = ctx.enter_context(tc.tile_pool(name="qspool", bufs=2))
        hpool = ctx.enter_context(tc.tile_pool(name="hpool", bufs=2, space="PSUM"))
        papool = ctx.enter_context(tc.tile_pool(name="papool", bufs=1, space="PSUM"))
        dzpool = ctx.enter_context(tc.tile_pool(name="dzpool", bufs=1, space="PSUM"))
        qpool = ctx.enter_context(tc.tile_pool(name="qpool", bufs=1, space="PSUM"))

        bt = wpool.tile([128, 2], F32)
        nc.sync.dma_start(bt[:], BC[:])
        wt = wpool.tile([128, 320], F16)
        nc.sync.dma_start(wt[:, 0:128], WC[:, 0:128])
        nc.sync.dma_start(wt[:, 128:320], WC[:, 128:320])
        w1 = wt[:, 0:128]
        pw1 = wt[:, 128:256]
        vw2 = wt[:, 256:288]
        pwt = wt[:, 288:320]
        bh = bt[:, 0:1]
        pb1 = bt[:, 1:2]

        pend = None  # (utA, utB, sq, col base) of the previous superblock

        def front_h(xt, utX, lo, hi):
            """4 composed K=32 h-matmuls + 2 fused tanh for one h-block."""
            h1 = hpool.tile([128, 1024], F32, tag="hps", name="hps")
            h2 = hpool.tile([128, 1024], F32, tag="hps", name="hps")
            for j, (ht, col) in enumerate(((h1, 0), (h1, 512), (h2, 0), (h2, 512))):
                nc.tensor.matmul(
                    ht[:, col : col + 512],
                    w1[32 * j : 32 * j + 32, :],
                    xt[32 * j : 32 * j + 32, lo:hi],
                    start=True, stop=True,
                    tile_position=(32 * j, 0),
                )
            nc.scalar.activation(utX[:, 0:1024], h1[:], AF.Tanh, bias=bh[:])
            nc.scalar.activation(utX[:, 1024:2048], h2[:], AF.Tanh, bias=bh[:])

        def front_pa(df, vt, which):
            """2 composed pa-matmuls + fused tanh for chunk pair 0 or 1."""
            pa = papool.tile([128, 1024], F32, tag="pa", name="pa")
            for k in range(2):
                j = 2 * which + k
                nc.tensor.matmul(
                    pa[:, 512 * k : 512 * k + 512],
                    pw1[32 * j : 32 * j + 32, :],
                    df[32 * j : 32 * j + 32, :],
                    start=True, stop=True,
                    tile_position=(32 * j, 0),
                )
            nc.scalar.activation(
                vt[:, 1024 * which : 1024 * which + 1024], pa[:], AF.Tanh, bias=pb1[:]
            )

        def back_dz(putX, pool=None):
            """dz for one h-block: 4 composed K=128/M=32 column-tiles."""
            # in the flush, dz-B borrows the qp slot (freed by the qs copy)
            # so it is not serialized behind combine-A's read of the dz slot
            dz = (pool or dzpool).tile(
                [128, 512], F32, tag="qp" if pool is not None else "dz",
                name="qp" if pool is not None else "dz")
            for j in range(4):
                nc.tensor.matmul(
                    dz[32 * j : 32 * j + 32, :],
                    vw2[:],
                    putX[:, 512 * j : 512 * (j + 1)],
                    start=True, stop=True,
                    tile_position=(0, 32 * j),
                )
            return dz

        def back_q(psq):
            qp = qpool.tile([128, 512], F32, tag="qp", name="qp")
            for j in range(4):
                nc.tensor.matmul(
                    qp[32 * j : 32 * j + 32, :],
                    pwt[:],
                    psq[:, 512 * j : 512 * (j + 1)],
                    start=True, stop=True,
                    tile_position=(0, 32 * j),
                )
            qs = qspool.tile([128, WPB], F32)
            nc.vector.tensor_copy(qs[:], qp[:])
            return qs

        def back_combine(dz, qs, pc0, half, flush=False):
            ot = opool.tile([128, WPB], F32, tag="ot", name="ot")
            op = OP.add if half == 0 else OP.subtract
            nc.vector.tensor_tensor(ot[:], dz[:], qs[:], op)
            # output DMA on the (otherwise idle) gpsimd queue so input
            # and output transfers use different hardware DMA queues; the
            # final (flush) blocks split across both queues to halve the
            # trailing transfer
            base = pc0 + WPB * half
            if flush:
                nc.sync.dma_start(O[:, base : base + WPB // 2], ot[:, : WPB // 2])
                nc.gpsimd.dma_start(O[:, base + WPB // 2 : base + WPB], ot[:, WPB // 2 :])
            else:
                nc.gpsimd.dma_start(O[:, base : base + WPB], ot[:])

        def back_all(putA, putB, psq, pc0, flush=False):
            dzA = back_dz(putA)
            qs = back_q(psq)
            back_combine(dzA, qs, pc0, 0, flush)
            dzB = back_dz(putB, pool=qpool if flush else None)
            back_combine(dzB, qs, pc0, 1, flush)

        for i in range(NSB + 1):
            if i < NSB:
                c0 = 2 * WPB * i
                xt = xpool.tile([128, 2 * WPB], F16)
                if i == 0:
                    # first x block: even half on the idle gpsimd queue, odd
                    # half behind the (tiny) weight DMAs on the sync queue,
                    # so both halves land in parallel
                    nc.gpsimd.dma_start(xt[:, 0:WPB], X[:, c0 : c0 + WPB])
                    nc.sync.dma_start(xt[:, WPB:], X[:, c0 + WPB : c0 + 2 * WPB])
                else:
                    nc.sync.dma_start(xt[:], X[:, c0 : c0 + 2 * WPB])
                df = dfpool.tile([128, WPB], F16)
                nc.vector.tensor_tensor(df[:], xt[:, 0:WPB], xt[:, WPB:], OP.subtract)
                utA = utpool.tile([128, 2048], F16, tag="ut", name="ut")
                utB = utpool.tile([128, 2048], F16, tag="ut", name="ut")
                vt = vtpool.tile([128, 2048], F16)
                if i < NSB - 1:
                    front_h(xt, utA, 0, 512)
                    front_pa(df, vt, 0)
                    front_h(xt, utB, 512, 1024)
                    if pend is not None:
                        back_all(*pend)
                    front_pa(df, vt, 1)
                    sq = sqpool.tile([128, 2048], F16)
                    nc.vector.tensor_mul(sq[:], vt[:], vt[:])
                else:
                    # last superblock: pair-net, the previous superblock's
                    # back-work, and this superblock's q/qs all run before
                    # the final h-block, so the flush only waits on the
                    # last h activation for dz-B
                    front_h(xt, utA, 0, 512)
                    front_pa(df, vt, 0)
                    front_pa(df, vt, 1)
                    sq = sqpool.tile([128, 2048], F16)
                    nc.vector.tensor_mul(sq[:], vt[:], vt[:])
                    back_all(*pend)
                    qs5 = back_q(sq)
                    front_h(xt, utB, 512, 1024)
                    pend = (utA, utB, qs5, c0)
                    continue
                pend = (utA, utB, sq, c0)
            else:
                putA, putB, qs, pc0 = pend
                dzA = back_dz(putA)
                back_combine(dzA, qs, pc0, 0, flush=True)
                dzB = back_dz(putB, pool=qpool)
                back_combine(dzB, qs, pc0, 1, flush=True)

    nc.compile()
    return nc


def _rowmaps():
    """col -> local row (0..6143) per chunk, and its inverse."""
    rowmap = np.empty(NCOL, dtype=np.int64)
    for b in range(NSB):
        i = np.arange(WPB)
        p = WPB * b + i
        rowmap[2 * WPB * b + i] = 2 * p
        rowmap[2 * WPB * b + WPB + i] = 2 * p + 1
    invmap = np.empty_like(rowmap)
    invmap[rowmap] = np.arange(NCOL)
    return rowmap, invmap


_ROWMAP, _INVMAP = _rowmaps()


def _prep_weights(t, vW1, vb1, vW2, vb2, pW1, pb1, pW2):
    f32 = np.float32
    t = np.asarray(t, dtype=f32).reshape(-1)[0]
    vW1 = np.asarray(vW1, dtype=f32)
    w1rep = np.tile(np.ascontiguousarray(vW1[:32]), (4, 1))            # [128,128]
    biash = (np.asarray(vb1, f32) + t * vW1[32]).reshape(128, 1).astype(f32)
    vw2 = np.ascontiguousarray(np.asarray(vW2, f32))                   # [128,32]
    pW1 = np.asarray(pW1, f32)
    pw1rep = np.tile(pW1, (4, 1))                                      # [128,128]
    pb1c = np.asarray(pb1, f32).reshape(128, 1).copy()
    w2col = np.asarray(pW2, f32).reshape(128)
    pw1tw2 = np.ascontiguousarray((pW1 * w2col[None, :]).T)            # [128,32]
    wcat = np.hstack([w1rep, pw1rep, vw2, pw1tw2]).astype(np.float16)
    bias = np.hstack([biash, pb1c]).astype(f32)
    # constant part of g: c0[d] = sum_k pW1[d,k]*w2[k], in the fp16 weight
    # precision actually used on device
    c0base = pw1tw2.astype(np.float16).astype(f32).sum(axis=0)         # [32]
    return {"wcat": np.ascontiguousarray(wcat), "bias": np.ascontiguousarray(bias),
            "_c0base": c0base}


def _pack_core(zc):
    """[25000, 32] f32 -> [128, 6144] fp16 packed: partition 32*j+d holds dim d
    of chunk j; chunk-local columns follow the even/odd superblock layout."""
    zp = zc.reshape(NCHUNK, CH, D)[:, :DROW, :]              # [4, 6144, 32]
    zp = zp[:, _ROWMAP, :]
    return zp.transpose(0, 2, 1).reshape(128, NCOL).astype(np.float16)


def _unpack_core(oc):
    """[128, 6144] packed f32 -> [4, 6144, 32] in chunk-local row order."""
    o = oc.reshape(NCHUNK, D, NCOL).transpose(0, 2, 1)       # [4, 6144, 32]
    return o[:, _INVMAP, :]


def _host_vnet(t, zr, vW1, vb1, vW2, vb2):
    f8 = np.float64
    t = float(np.asarray(t).reshape(-1)[0])
    vW1 = np.asarray(vW1, f8)
    h = np.tanh(zr.astype(f8) @ vW1[:32] + t * vW1[32] + np.asarray(vb1, f8))
    return h @ np.asarray(vW2, f8) + np.asarray(vb2, f8)


def _host_pairs(t, zE, zO, vW1, vb1, vW2, vb2, pW1, pb1, pW2):
    """Exact v-net + pair force for leftover pairs: returns (outE, outO)."""
    f8 = np.float64
    pW1 = np.asarray(pW1, f8)
    w2 = np.asarray(pW2, f8).reshape(128)
    d = zE.astype(f8) - zO.astype(f8)
    u = np.tanh(d @ pW1 + np.asarray(pb1, f8))
    g = ((1.0 - u * u) * w2[None, :]) @ pW1.T               # grad_phi rows
    outE = _host_vnet(t, zE, vW1, vb1, vW2, vb2) - g
    outO = _host_vnet(t, zO, vW1, vb1, vW2, vb2) + g
    return outE.astype(np.float32), outO.astype(np.float32)


def _host_triple(t, z3, vW1, vb1, vW2, vb2, pW1, pb1, pW2):
    """Exact float64 computation of the 3 leftover rows: dz_dt + triple forces."""
    f8 = np.float64
    pW1 = np.asarray(pW1, f8)
    w2 = np.asarray(pW2, f8).reshape(128)
    z3 = z3.astype(f8)
    d9 = (z3[:, None, :] - z3[None, :, :]).reshape(9, 32)
    u9 = np.tanh(d9 @ pW1 + np.asarray(pb1, f8))
    s9 = (1.0 - u9 * u9) * w2[None, :]
    g9 = s9 @ pW1.T                       # grad_phi rows
    f9 = (-g9).reshape(3, 3, 32)
    f9 = f9 * (1.0 - np.eye(3)[:, :, None])
    force3 = f9.sum(axis=1) * 2.0
    return (_host_vnet(t, z3, vW1, vb1, vW2, vb2) + force3).astype(np.float32)


def kernel(t, z, perm, vW1, vb1, vW2, vb2, pW1, pb1, pW2, pb2):
    from concourse.bass_utils import run_bass_kernel_spmd

    global LAST_RESULTS
    if "nc" not in _CACHE:
        _CACHE["nc"] = build_program()
    nc = _CACHE["nc"]

    z = np.asarray(z, np.float32)
    perm = np.asarray(perm)
    weights = _prep_weights(t, vW1, vb1, vW2, vb2, pW1, pb1, pW2)

    c0base = weights.pop("_c0base")
    zg = z[perm[:P2]]                       # [200000, 32] gathered pair rows
    in_maps = []
    for c in range(NCORES):
        im = {"x": _pack_core(zg[c * RPC : (c + 1) * RPC])}
        im.update(weights)
        in_maps.append(im)

    trace = bool(int(os.environ.get("KERNEL_TRACE", "0")))
    res = run_bass_kernel_spmd(nc, in_maps, list(range(NCORES)), trace=trace)
    LAST_RESULTS = res

    out = np.empty((B, 32), dtype=np.float32)
    og = np.empty((P2, D), dtype=np.float32)
    vb2f = np.asarray(vb2, np.float32)
    dev_even = (vb2f - c0base)[None, :]
    dev_odd = (vb2f + c0base)[None, :]
    # leftover rows (local rows DROW..CH-1 of each chunk) computed on host
    lrow = np.arange(DROW, CH)
    lE = lrow[0::2]
    lO = lrow[1::2]
    for c in range(NCORES):
        od = _unpack_core(res.results[c]["out"])             # [4, 6144, 32]
        zc = zg[c * RPC : (c + 1) * RPC].reshape(NCHUNK, CH, D)
        for j in range(NCHUNK):
            base = c * RPC + j * CH
            blk = og[base : base + CH]
            blk[:DROW] = od[j]
            blk[:DROW:2] += dev_even
            blk[1:DROW:2] += dev_odd
            blk[lE], blk[lO] = _host_pairs(
                t, zc[j, lE], zc[j, lO], vW1, vb1, vW2, vb2, pW1, pb1, pW2
            )
    out[perm[:P2]] = og
    out[perm[P2:]] = _host_triple(t, z[perm[P2:]], vW1, vb1, vW2, vb2, pW1, pb1, pW2)
    return out
